# revision 1
# baseline (speedup 1.0000x reference)
"""Multi-head attention (B=2, S=2048, D=1024, H=16) on 8 TRN2 NeuronCores.

Sharding: batch x head-group. Core c handles batch b=c//4 and heads
[4g, 4g+4) with g=c%4 (column-parallel QKV projections, row-parallel
output projection). Each core emits a partial [S, D] output; the host
sums the 4 partials per batch (the row-parallel all-reduce).

Device-side dataflow per core (all matmuls bf16 with f32 PSUM accum):
  qhT/khT [p=256, s] = Wg.T-weighted projections of q/k (q pre-scaled
  by 1/sqrt(hd) on host); vh [s, p] likewise, augmented with a ones
  column per head so the attention row-sums fall out of the AV matmul.
  Scores are computed transposed (scoresT[j, i]) so softmax renormali-
  zation and the AV contraction both run without any on-chip transpose:
  exp via ScalarE straight out of PSUM, causal masking via a single
  [128,128] additive bias tile on the diagonal blocks, and strictly-
  upper blocks are never computed.
"""

import os
import numpy as np
import ml_dtypes

import concourse.bass as bass
import concourse.tile as tile
from concourse import bacc, mybir
from concourse.bass_utils import run_bass_kernel_spmd

B, S, D, H = 2, 2048, 1024, 16
HD = D // H          # 64
HL = H // 4          # 4 heads per core
PL = HL * HD         # 256 local projection dim
KT = D // 128        # 8 contraction blocks
SB = S // 128        # 16 sequence blocks of 128
CH = S // 512        # 4 sequence chunks of 512
F32 = mybir.dt.float32
DT = mybir.dt.bfloat16
NP_DT = ml_dtypes.bfloat16

_cache = {}
last_results = None


def build_program():
    if "nc" in _cache:
        return _cache["nc"]
    nc = bacc.Bacc("TRN2", target_bir_lowering=False, debug=False, num_devices=8)

    qt_d = nc.dram_tensor("qt", [D, S], DT, kind="ExternalInput")
    kt_d = nc.dram_tensor("kt", [D, S], DT, kind="ExternalInput")
    vt_d = nc.dram_tensor("vt", [D, S], DT, kind="ExternalInput")
    # weights arrive pre-arranged in their SBUF layouts so every DMA is a
    # straight contiguous copy (strided rearrange DMAs cost ~20us of tiny
    # descriptors before the first matmul can start)
    wq_d = nc.dram_tensor("wq", [128, KT, PL], DT, kind="ExternalInput")
    wk_d = nc.dram_tensor("wk", [128, KT, PL], DT, kind="ExternalInput")
    wv_d = nc.dram_tensor("wv", [128, KT, PL], DT, kind="ExternalInput")
    wf_d = nc.dram_tensor("wf", [128, 2, D], DT, kind="ExternalInput")
    bq_d = nc.dram_tensor("bq2", [128, 2], F32, kind="ExternalInput")
    bk_d = nc.dram_tensor("bk2", [128, 2], F32, kind="ExternalInput")
    bv_d = nc.dram_tensor("bv1", [1, PL], F32, kind="ExternalInput")
    bf_d = nc.dram_tensor("bf1", [1, D], F32, kind="ExternalInput")
    tri_d = nc.dram_tensor("tri", [128, 128], F32, kind="ExternalInput")
    sel_d = nc.dram_tensor("sel", [40, 16 * HD], mybir.dt.float16, kind="ExternalInput")
    out_d = nc.dram_tensor("out", [S, D], DT, kind="ExternalOutput")

    ADD = mybir.AluOpType.add
    MUL = mybir.AluOpType.mult
    EXP = mybir.ActivationFunctionType.Exp

    with tile.TileContext(nc) as tc:
        with (
            tc.tile_pool(name="singles", bufs=1) as singles,
            tc.tile_pool(name="inp", bufs=12) as inp,
            tc.tile_pool(name="epool", bufs=6) as epool,
            tc.tile_pool(name="apool", bufs=2) as apool,
            tc.tile_pool(name="opool", bufs=3) as opool,
            tc.tile_pool(name="psum", bufs=2, space="PSUM") as psum,
        ):
            wq_sb = singles.tile([128, KT, PL], DT)
            wk_sb = singles.tile([128, KT, PL], DT)
            wv_sb = singles.tile([128, KT, PL], DT)
            wf_sb = singles.tile([128, 2, D], DT)
            bq_sb = singles.tile([128, 2], F32)
            bk_sb = singles.tile([128, 2], F32)
            bv_bc = singles.tile([128, PL], F32)
            bf_bc = singles.tile([128, D], F32)
            tri_sb = singles.tile([128, 128], F32)

            qhT = singles.tile([128, 2, S], DT)   # [p within block, pblock, s]
            khT = singles.tile([128, 2, S], DT)
            vh = singles.tile([128, SB, HL, HD + 1], DT)  # [s within blk, sblk, h, hd|ones]
            xn = singles.tile([128, 2, S], DT)    # normalized attn out, head pairs stacked
            nc.vector.memset(vh[:, :, :, HD : HD + 1], 1.0)
            sel_sb = singles.tile([40, 16 * HD], mybir.dt.float16)
            nc.sync.dma_start(sel_sb, sel_d.ap())
            sums_sb = singles.tile([40, 512], F32)   # row 32*hb + 4*par + c = sums of (h, c)
            rsum16 = singles.tile([40, 512], mybir.dt.float16)
            nc.vector.memset(rsum16, 0.0)  # one-hot bcast matmuls read all 16 rows

            # ---- Phase B: projections ----
            def load_blocks(x_d):
                ts = []
                for kk in range(KT):
                    t = inp.tile([128, S], DT, tag="xin")
                    nc.sync.dma_start(t, x_d.ap()[128 * kk : 128 * (kk + 1), :])
                    ts.append(t)
                return ts

            def proj_ph(xt, w_sb, b_sb, out_sb):
                for pt in range(2):
                    for ch in range(CH):
                        pp = psum.tile([128, 512], F32, tag="A", bufs=3)
                        for kk in range(KT):
                            nc.tensor.matmul(
                                pp,
                                lhsT=w_sb[:, kk, 128 * pt : 128 * (pt + 1)],
                                rhs=xt[kk][:, 512 * ch : 512 * (ch + 1)],
                                start=(kk == 0),
                                stop=(kk == KT - 1),
                            )
                        nc.vector.tensor_scalar_add(
                            out_sb[:, pt, 512 * ch : 512 * (ch + 1)],
                            pp,
                            b_sb[:, pt : pt + 1],
                        )

            qt_t = load_blocks(qt_d)
            nc.sync.dma_start(wq_sb, wq_d.ap())
            nc.sync.dma_start(bq_sb, bq_d.ap())
            proj_ph(qt_t, wq_sb, bq_sb, qhT)
            kt_t = load_blocks(kt_d)
            nc.sync.dma_start(wk_sb, wk_d.ap())
            nc.sync.dma_start(bk_sb, bk_d.ap())
            proj_ph(kt_t, wk_sb, bk_sb, khT)
            vt_t = load_blocks(vt_d)
            nc.sync.dma_start(wv_sb, wv_d.ap())
            nc.sync.dma_start(bv_bc, bv_d.ap().to_broadcast([128, PL]))
            nc.sync.dma_start(tri_sb, tri_d.ap())
            nc.sync.dma_start(wf_sb, wf_d.ap())
            nc.sync.dma_start(bf_bc, bf_d.ap().to_broadcast([128, D]))
            for sb in range(SB):
                pv = psum.tile([128, PL], F32, tag="B", bufs=2)
                for kk in range(KT):
                    nc.tensor.matmul(
                        pv,
                        lhsT=vt_t[kk][:, 128 * sb : 128 * (sb + 1)],
                        rhs=wv_sb[:, kk, :],
                        start=(kk == 0),
                        stop=(kk == KT - 1),
                    )
                nc.vector.tensor_tensor(
                    out=vh[:, sb, :, 0:HD],
                    in0=pv.rearrange("p (h e) -> p h e", h=HL),
                    in1=bv_bc.rearrange("p (h e) -> p h e", h=HL),
                    op=ADD,
                )

            # ---- Phase C: attention, head pairs interleaved ----
            # Heads 2*hb (partitions 0:64 of block hb) and 2*hb+1 (64:128)
            # issue back-to-back K=64 score matmuls whose lhsT base
            # partitions auto-derive tile_position (0,0) / (64,0): they run
            # concurrently in the PE array and land in separate PSUM banks
            # of the same pair tile, which one ScalarE exp covers whole.
            xas = {}

            def emit_normalize(hb):
                # softmax renormalization for head pair hb: one batched
                # reciprocal over its 8 row-sum rows, then a per-(h,c)
                # partition-broadcast of the reciprocal row via a one-hot
                # fp16 selector matmul (gpsimd partition_broadcast is broken
                # on HW; step-0-partition DMA from SBUF is rejected; single-
                # partition DVE reciprocals cost 3.3us each). Emitted
                # interleaved into the next phase's code so the in-order PE
                # stream is not stalled on the DVE reciprocal.
                nc.vector.reciprocal(
                    sums_sb[32 * hb : 32 * hb + 8, :],
                    sums_sb[32 * hb : 32 * hb + 8, :],
                )
                nc.vector.tensor_copy(
                    rsum16[32 * hb : 32 * hb + 8, :],
                    sums_sb[32 * hb : 32 * hb + 8, :],
                )
                for par in range(2):
                    h = 2 * hb + par
                    ro = 64 * par
                    for c in range(CH):
                        hc = 4 * h + c
                        rb = psum.tile([HD, 512], F32, tag="B", bufs=2)
                        nc.tensor.matmul(
                            rb,
                            lhsT=sel_sb[:, HD * hc : HD * (hc + 1)],
                            rhs=rsum16,
                            start=True,
                            stop=True,
                        )
                        xt_n = apool.tile([HD, 512], DT, tag="xtn")
                        nc.vector.tensor_tensor(
                            out=xt_n, in0=xas[hc][0:HD, :], in1=rb, op=MUL
                        )
                        nc.sync.dma_start(
                            xn[ro : ro + 64, hb, 512 * c : 512 * (c + 1)], xt_n
                        )

            for hb in range(2):
                for c in range(CH):
                    if hb == 1 and c == 1:
                        emit_normalize(0)
                    pxs = [
                        psum.tile([128, 512], F32, tag="B", bufs=2, name=f"px{p_}")
                        for p_ in range(2)
                    ]
                    nbj = 4 * c + 4
                    for bj in range(nbj):
                        band = bj >= 4 * c
                        i0 = 128 * bj if band else 512 * c
                        w = 512 * (c + 1) - i0
                        o = i0 - 512 * c
                        psp = psum.tile([128, 2, 512], F32, tag="A", bufs=3)
                        for par in range(2):
                            nc.tensor.matmul(
                                psp[:, par, 0:w],
                                lhsT=khT[
                                    64 * par : 64 * par + 64,
                                    hb,
                                    128 * bj : 128 * (bj + 1),
                                ],
                                rhs=qhT[64 * par : 64 * par + 64, hb, i0 : i0 + w],
                                start=True,
                                stop=True,
                            )
                        if band:
                            for par in range(2):
                                nc.vector.tensor_tensor(
                                    out=psp[:, par, 0:128],
                                    in0=psp[:, par, 0:128],
                                    in1=tri_sb,
                                    op=ADD,
                                )
                        et = epool.tile([128, 2, 512], DT, tag="et")
                        nc.scalar.activation(et[:, :, 0:w], psp[:, :, 0:w], EXP)
                        for par in range(2):
                            nc.tensor.matmul(
                                pxs[par][0 : HD + 1, o : o + w],
                                lhsT=vh[:, bj, 2 * hb + par, :],
                                rhs=et[:, par, 0:w],
                                start=(bj == 0),
                                stop=(bj == nbj - 1),
                            )
                    for par in range(2):
                        hc = 4 * (2 * hb + par) + c
                        xa = apool.tile([HD + 1, 512], F32, tag="xa", bufs=16)
                        nc.vector.tensor_copy(out=xa, in_=pxs[par][0 : HD + 1, :])
                        row = 32 * hb + 4 * par + c
                        nc.sync.dma_start(sums_sb[row : row + 1, :], xa[HD : HD + 1, :])
                        xas[hc] = xa

            emit_normalize(1)

            # ---- Phase D: output projection (partial; host sums over groups) ----
            for ib in range(SB):
                for oc in range(2):
                    po = psum.tile([128, 512], F32, tag="A", bufs=3)
                    for t in range(2):
                        nc.tensor.matmul(
                            po,
                            lhsT=xn[:, t, 128 * ib : 128 * (ib + 1)],
                            rhs=wf_sb[:, t, 512 * oc : 512 * (oc + 1)],
                            start=(t == 0),
                            stop=(t == 1),
                        )
                    ob = opool.tile([128, 512], DT, tag="ob")
                    nc.vector.tensor_tensor(
                        out=ob, in0=po, in1=bf_bc[:, 512 * oc : 512 * (oc + 1)], op=ADD
                    )
                    nc.sync.dma_start(
                        out_d.ap()[128 * ib : 128 * (ib + 1), 512 * oc : 512 * (oc + 1)],
                        ob,
                    )

    nc.compile()
    _cache["nc"] = nc
    return nc


def _wlayout(wT):
    # [D, PL] -> SBUF layout [128, KT, PL]
    return np.ascontiguousarray(wT.reshape(KT, 128, PL).transpose(1, 0, 2)).astype(NP_DT)


def _flayout(wT):
    # [PL, D] -> SBUF layout [128, 2, D]
    return np.ascontiguousarray(wT.reshape(2, 128, D).transpose(1, 0, 2)).astype(NP_DT)


def make_in_maps(q, k, v, mask, Wq, bq, Wk, bk, Wv, bv, Wf, bf):
    scale = 1.0 / np.sqrt(np.float32(HD))
    f32 = np.float32
    m = np.asarray(mask[0, 0])
    tri = np.where(m[:128, :128].T == 0, f32(-1e9), f32(0.0)).astype(f32)
    sel = np.zeros((40, 16 * HD), np.float16)
    for hc in range(16):
        sel[32 * (hc // 8) + hc % 8, HD * hc : HD * (hc + 1)] = 1.0
    in_maps = []
    for c in range(8):
        b, g = c // 4, c % 4
        sl = slice(g * PL, (g + 1) * PL)
        in_maps.append(
            {
                "qt": np.ascontiguousarray((np.asarray(q[b]).T * scale)).astype(NP_DT),
                "kt": np.ascontiguousarray(np.asarray(k[b]).T).astype(NP_DT),
                "vt": np.ascontiguousarray(np.asarray(v[b]).T).astype(NP_DT),
                "wq": _wlayout(np.asarray(Wq)[sl, :].T),
                "wk": _wlayout(np.asarray(Wk)[sl, :].T),
                "wv": _wlayout(np.asarray(Wv)[sl, :].T),
                "wf": _flayout(np.asarray(Wf)[:, sl].T),
                "bq2": np.ascontiguousarray((np.asarray(bq)[sl] * scale).astype(f32).reshape(2, 128).T),
                "bk2": np.ascontiguousarray(np.asarray(bk)[sl].astype(f32).reshape(2, 128).T),
                "bv1": np.asarray(bv)[sl].astype(f32).reshape(1, PL),
                "bf1": (np.asarray(bf).astype(f32) / 4.0).reshape(1, D),
                "tri": tri,
                "sel": sel,
            }
        )
    return in_maps


def _mask_is_causal(mask):
    m = np.asarray(mask[0, 0])
    return bool(np.array_equal(m != 0, np.tril(np.ones((S, S), bool))))


def _numpy_fallback(q, k, v, mask, Wq, bq, Wk, bk, Wv, bv, Wf, bf):
    out = np.empty((B, S, D), np.float32)
    m = np.asarray(mask[0, 0])
    for b in range(B):
        qh = (np.asarray(q[b]) @ np.asarray(Wq).T + bq).reshape(S, H, HD)
        kh = (np.asarray(k[b]) @ np.asarray(Wk).T + bk).reshape(S, H, HD)
        vh = (np.asarray(v[b]) @ np.asarray(Wv).T + bv).reshape(S, H, HD)
        x = np.empty((S, H, HD), np.float32)
        for hh in range(H):
            sc = qh[:, hh] @ kh[:, hh].T / np.sqrt(np.float32(HD))
            sc = np.where(m == 0, np.float32(-1e9), sc)
            sc = sc - sc.max(-1, keepdims=True)
            e = np.exp(sc)
            x[:, hh] = (e / e.sum(-1, keepdims=True)) @ vh[:, hh]
        out[b] = x.reshape(S, D) @ np.asarray(Wf).T + bf
    return out


def kernel(q, k, v, mask, Wq, bq, Wk, bk, Wv, bv, Wf, bf):
    global last_results
    if not _mask_is_causal(mask):
        return _numpy_fallback(q, k, v, mask, Wq, bq, Wk, bk, Wv, bv, Wf, bf)
    nc = build_program()
    in_maps = make_in_maps(q, k, v, mask, Wq, bq, Wk, bk, Wv, bv, Wf, bf)
    res = run_bass_kernel_spmd(nc, in_maps, core_ids=list(range(8)))
    last_results = res
    out = np.zeros((B, S, D), np.float32)
    for c in range(8):
        out[c // 4] += res.results[c]["out"].astype(np.float32)
    return out



# revision 9
# speedup vs baseline: 1.1152x; 1.1152x over previous
"""Multi-head attention (B=2, S=2048, D=1024, H=16) on 8 TRN2 NeuronCores.

Sharding: batch x head-group. Core c handles batch b=c//4 and heads
[4g, 4g+4) with g=c%4 (column-parallel QKV projections, row-parallel
output projection). Each core emits a partial [S, D] output; the host
sums the 4 partials per batch (the row-parallel all-reduce).

Chunk-major pipeline (v2): the sequence is processed in 4 chunks of 512
queries. Per chunk: project q/k for that chunk (+v for its key blocks),
run both head-pairs' scores->exp->AV, normalize, and do the output
projection — so softmax normalization and the out-projection of chunk c
hide under the attention of chunk c+1 instead of forming a serial tail.
Host re-layouts q/k/v so every chunk's input is one contiguous DMA.

Other key choices (all matmuls bf16 with f32 PSUM accum):
- Scores are computed transposed (scoresT[k, q]); the K=64 head-pair
  matmuls run concurrently in the PE array via base-partition row
  tiling. vh carries a ones column so attention row-sums fall out of
  the AV matmul.
- Causal masking of diagonal blocks is a PE preload: a [128,128]
  additive -1e9 matmul with start=True, which the score matmul then
  accumulates onto (cols beyond 128 are overwritten since their
  has_written bits stay clear) — no VectorE masking pass.
- Softmax renormalization: row sums gathered to a [4,512] tile, 1/x on
  ScalarE as exp(-ln(x)) (both fns in one ACT table set), broadcast
  across partitions by a tiny one-hot fp16 matmul, applied by VectorE.
- ~80 junk warmup matmuls at t=0 keep the PE HAM clock-gate warm while
  the first input DMAs land.
"""

import os
import numpy as np
import ml_dtypes

import concourse.bass as bass
import concourse.tile as tile
from concourse import bacc, mybir
from concourse.bass_utils import run_bass_kernel_spmd

B, S, D, H = 2, 2048, 1024, 16
HD = D // H          # 64
HL = H // 4          # 4 heads per core
PL = HL * HD         # 256 local projection dim
KT = D // 128        # 8 contraction blocks
SB = S // 128        # 16 sequence blocks of 128
CH = S // 512        # 4 sequence chunks of 512
F32 = mybir.dt.float32
F16 = mybir.dt.float16
DT = mybir.dt.bfloat16
NP_DT = ml_dtypes.bfloat16
USE_ACT_RECIP = False  # Ln+Exp thrash ACT table sets (9 loads); DVE recip hides under the skew

_cache = {}
last_results = None


def build_program():
    if "nc" in _cache:
        return _cache["nc"]
    nc = bacc.Bacc("TRN2", target_bir_lowering=False, debug=False, num_devices=8)

    # inputs, host-relaid so every DMA is contiguous with >=2KB/partition
    qc_d = nc.dram_tensor("qc", [CH, 128, KT, 512], DT, kind="ExternalInput")
    kc_d = nc.dram_tensor("kc", [CH, 128, KT, 512], DT, kind="ExternalInput")
    vc_d = nc.dram_tensor("vc", [SB, 128, KT, 128], DT, kind="ExternalInput")
    wq_d = nc.dram_tensor("wq", [128, KT, PL], DT, kind="ExternalInput")
    wk_d = nc.dram_tensor("wk", [128, KT, PL], DT, kind="ExternalInput")
    wv_d = nc.dram_tensor("wv", [128, KT, PL], DT, kind="ExternalInput")
    wf_d = nc.dram_tensor("wf", [128, 2, D], DT, kind="ExternalInput")
    bq_d = nc.dram_tensor("bq2", [128, 2], F32, kind="ExternalInput")
    bk_d = nc.dram_tensor("bk2", [128, 2], F32, kind="ExternalInput")
    bv_d = nc.dram_tensor("bv1", [1, PL], F32, kind="ExternalInput")
    bf_d = nc.dram_tensor("bf1", [1, D], F32, kind="ExternalInput")
    tri_d = nc.dram_tensor("tri", [128, 128], DT, kind="ExternalInput")
    idn_d = nc.dram_tensor("idn", [128, 128], DT, kind="ExternalInput")
    sel_d = nc.dram_tensor("sel", [4, 4 * HD], F16, kind="ExternalInput")
    out_d = nc.dram_tensor("out", [S, D], DT, kind="ExternalOutput")

    ADD = mybir.AluOpType.add
    MUL = mybir.AluOpType.mult
    EXP = mybir.ActivationFunctionType.Exp
    LN = mybir.ActivationFunctionType.Ln

    with tile.TileContext(nc) as tc:
        with (
            tc.tile_pool(name="singles", bufs=1) as singles,
            tc.tile_pool(name="qk", bufs=3) as qkp,
            tc.tile_pool(name="vin", bufs=6) as vinp,
            tc.tile_pool(name="epool", bufs=6) as epool,
            tc.tile_pool(name="apool", bufs=6) as apool,
            tc.tile_pool(name="npool", bufs=2) as npool,
            tc.tile_pool(name="opool", bufs=2) as opool,
            tc.tile_pool(name="psum", bufs=2, space="PSUM") as psum,
        ):
            wq_sb = singles.tile([128, KT, PL], DT)
            wk_sb = singles.tile([128, KT, PL], DT)
            wv_sb = singles.tile([128, KT, PL], DT)
            wf_sb = singles.tile([128, 2, D], DT)
            bq_sb = singles.tile([128, 2], F32)
            bk_sb = singles.tile([128, 2], F32)
            bv_bc = singles.tile([128, PL], F32)
            bf_bc = singles.tile([128, D], F32)
            tri_sb = singles.tile([128, 128], DT)
            idn_sb = singles.tile([128, 128], DT)
            sel_sb = singles.tile([4, 4 * HD], F16)

            qhT = singles.tile([128, 2, S], DT)   # [p within pair, pair, s]
            khT = singles.tile([128, 2, S], DT)
            vh = singles.tile([128, SB, HL, HD + 1], DT)  # [s in blk, sblk, h, hd|1]
            xn = singles.tile([128, 2, S], DT)    # normalized attn out
            nc.vector.memset(vh[:, :, :, HD : HD + 1], 1.0)

            # warmup matmuls: keep the PE HAM activity window busy while
            # the first input DMAs land, so projections start at 2.4 GHz
            wu = singles.tile([128, 128], DT)
            nc.vector.memset(wu, 0.0)
            for _ in range(80):
                wp = psum.tile([128, 128], F32, tag="C", bufs=2, name="wu")
                nc.tensor.matmul(wp, lhsT=wu, rhs=wu, start=True, stop=True)

            # weights first (small), then per-chunk inputs
            nc.sync.dma_start(wq_sb, wq_d.ap())
            nc.sync.dma_start(bq_sb, bq_d.ap())
            nc.sync.dma_start(wk_sb, wk_d.ap())
            nc.sync.dma_start(bk_sb, bk_d.ap())
            nc.sync.dma_start(wv_sb, wv_d.ap())
            nc.sync.dma_start(tri_sb, tri_d.ap())
            nc.sync.dma_start(idn_sb, idn_d.ap())
            nc.sync.dma_start(bv_bc, bv_d.ap().to_broadcast([128, PL]))

            # DMA issue is decoupled from compute: inputs for chunk c+1 are
            # DMA'd at the start of chunk c, and their projections are
            # emitted as PE "filler" between attention blocks of chunk c
            # (the PE queue is strictly in-order, so without filler it
            # would idle every block waiting on ScalarE's exp).
            def issue_qk(c):
                xq = qkp.tile([128, KT, 512], DT, tag="xin")
                nc.sync.dma_start(xq, qc_d.ap()[c])
                xk = qkp.tile([128, KT, 512], DT, tag="xin")
                nc.sync.dma_start(xk, kc_d.ap()[c])
                return xq, xk

            def issue_v(c):
                vts = []
                for sb in range(4 * c, 4 * c + 4):
                    vt = vinp.tile([128, KT, 128], DT, tag="vin", bufs=8)
                    nc.sync.dma_start(vt, vc_d.ap()[sb])
                    vts.append(vt)
                return vts

            def proj_item(xt, c, pt, w_sb, b_sb, out_sb):
                pp = psum.tile([128, 512], F32, tag="C", bufs=2, name="pp")
                for kk in range(KT):
                    nc.tensor.matmul(
                        pp,
                        lhsT=w_sb[:, kk, 128 * pt : 128 * (pt + 1)],
                        rhs=xt[:, kk, :],
                        start=(kk == 0),
                        stop=(kk == KT - 1),
                    )
                nc.vector.tensor_scalar_add(
                    out_sb[:, pt, 512 * c : 512 * (c + 1)],
                    pp,
                    b_sb[:, pt : pt + 1],
                )

            def vproj_item(vt, sb):
                pv = psum.tile([128, PL], F32, tag="C", bufs=2, name="pv")
                for kk in range(KT):
                    nc.tensor.matmul(
                        pv,
                        lhsT=vt[:, kk, :],
                        rhs=wv_sb[:, kk, :],
                        start=(kk == 0),
                        stop=(kk == KT - 1),
                    )
                nc.vector.tensor_tensor(
                    out=vh[:, sb, :, 0:HD],
                    in0=pv.rearrange("p (h e) -> p h e", h=HL),
                    in1=bv_bc.rearrange("p (h e) -> p h e", h=HL),
                    op=ADD,
                )

            def norm_item(c, r, xas, rec16):
                hb, par = r // 2, r % 2
                rb = psum.tile([HD, 512], F32, tag="C", bufs=2, name="rb")
                nc.tensor.matmul(
                    rb,
                    lhsT=sel_sb[:, HD * r : HD * (r + 1)],
                    rhs=rec16,
                    start=True,
                    stop=True,
                )
                xt_n = npool.tile([HD, 512], DT, tag="xtn", bufs=4)
                nc.vector.tensor_tensor(out=xt_n, in0=xas[r][0:HD, :], in1=rb, op=MUL)
                nc.sync.dma_start(
                    xn[64 * par : 64 * par + 64, hb, 512 * c : 512 * (c + 1)], xt_n
                )

            def d_item(c, ib):
                ob = opool.tile([128, D], DT, tag="ob")
                for oc in range(2):
                    po = psum.tile([128, 512], F32, tag="C", bufs=2, name="po")
                    for t in range(2):
                        nc.tensor.matmul(
                            po,
                            lhsT=xn[:, t, 128 * ib : 128 * (ib + 1)],
                            rhs=wf_sb[:, t, 512 * oc : 512 * (oc + 1)],
                            start=(t == 0),
                            stop=(t == 1),
                        )
                    nc.vector.tensor_tensor(
                        out=ob[:, 512 * oc : 512 * (oc + 1)],
                        in0=po,
                        in1=bf_bc[:, 512 * oc : 512 * (oc + 1)],
                        op=ADD,
                    )
                nc.sync.dma_start(out_d.ap()[128 * ib : 128 * (ib + 1), :], ob)

            def attn_chunk(c, filler):
                """Attention for chunk c, interleaving `filler` PE work into
                the exp-latency gaps. AV(bj) is emitted one block late so
                scores(bj+1)+filler cover exp(bj)'s ScalarE latency."""
                xas = {}
                sums = npool.tile([4, 512], F32, tag="sums", bufs=3)
                nbj = 4 * c + 4
                slots = 2 * nbj

                def pop_filler():
                    nonlocal slots
                    n = (len(filler) + slots - 1) // slots if slots > 0 else len(filler)
                    for _ in range(min(n, len(filler))):
                        filler.pop(0)()
                    slots -= 1

                for hb in range(2):
                    pxs = [
                        psum.tile([128, 512], F32, tag="B", bufs=2, name=f"px{p_}")
                        for p_ in range(2)
                    ]
                    pend_av = None
                    for bj in range(nbj):
                        band = bj >= 4 * c
                        i0 = 128 * bj if band else 512 * c
                        w = 512 * (c + 1) - i0
                        o = i0 - 512 * c
                        psp = psum.tile([128, 2, 512], F32, tag="A", bufs=2)
                        for par in range(2):
                            nc.tensor.matmul(
                                psp[:, par, 0:w],
                                lhsT=khT[
                                    64 * par : 64 * par + 64,
                                    hb,
                                    128 * bj : 128 * (bj + 1),
                                ],
                                rhs=qhT[64 * par : 64 * par + 64, hb, i0 : i0 + w],
                                start=True,
                                stop=not band,
                            )
                        if band:
                            # causal masking on the PE: accumulate a -1e9
                            # upper-triangle matmul onto the diagonal
                            # 128x128 sub-block (cols 0:128)
                            for par in range(2):
                                nc.tensor.matmul(
                                    psp[:, par, 0:128],
                                    lhsT=tri_sb,
                                    rhs=idn_sb,
                                    start=False,
                                    stop=True,
                                )
                        et = epool.tile([128, 2, 512], DT, tag="et")
                        nc.scalar.activation(et[:, :, 0:w], psp[:, :, 0:w], EXP)
                        if pend_av is not None:
                            pop_filler()
                            pend_av()
                        pend_av = (
                            lambda et=et, w=w, o=o, bj=bj: [
                                nc.tensor.matmul(
                                    pxs[par][0 : HD + 1, o : o + w],
                                    lhsT=vh[:, bj, 2 * hb + par, :],
                                    rhs=et[:, par, 0:w],
                                    start=(bj == 0),
                                    stop=(bj == nbj - 1),
                                )
                                for par in range(2)
                            ]
                        )
                    pop_filler()
                    pend_av()
                    for par in range(2):
                        r = 2 * hb + par
                        xa = apool.tile([HD + 1, 512], F32, tag="xa", bufs=10)
                        nc.vector.tensor_copy(out=xa, in_=pxs[par][0 : HD + 1, :])
                        nc.sync.dma_start(sums[r : r + 1, :], xa[HD : HD + 1, :])
                        xas[r] = xa
                rec16 = npool.tile([4, 512], F16, tag="rec16")
                if USE_ACT_RECIP:
                    lg = npool.tile([4, 512], F32, tag="lg")
                    nc.scalar.activation(lg, sums, LN)
                    rec = npool.tile([4, 512], F32, tag="rec")
                    nc.scalar.activation(rec, lg, EXP, scale=-1.0)
                else:
                    rec = npool.tile([4, 512], F32, tag="rec")
                    nc.vector.reciprocal(rec, sums)
                nc.vector.tensor_copy(rec16, rec)
                # drain any leftover filler
                for f in filler:
                    f()
                return xas, rec16

            # ---- prologue: chunk 0 inputs + projections, serial ----
            xq0, xk0 = issue_qk(0)
            vts0 = issue_v(0)
            for pt in range(2):
                proj_item(xq0, 0, pt, wq_sb, bq_sb, qhT)
            for pt in range(2):
                proj_item(xk0, 0, pt, wk_sb, bk_sb, khT)
            for j, sb in enumerate(range(0, 4)):
                vproj_item(vts0[j], sb)

            pending = None
            for c in range(CH):
                filler = []
                if c + 1 < CH:
                    xq, xk = issue_qk(c + 1)
                    vts = issue_v(c + 1)
                    if c == 0:
                        nc.sync.dma_start(wf_sb, wf_d.ap())
                        nc.sync.dma_start(bf_bc, bf_d.ap().to_broadcast([128, D]))
                        nc.sync.dma_start(sel_sb, sel_d.ap())
                if pending is not None:
                    pc, pxas, prec16 = pending
                    for r in range(4):
                        filler.append(
                            lambda r=r: norm_item(pc, r, pxas, prec16)
                        )
                if c + 1 < CH:
                    for pt in range(2):
                        filler.append(
                            lambda pt=pt, xq=xq, cn=c + 1: proj_item(
                                xq, cn, pt, wq_sb, bq_sb, qhT
                            )
                        )
                    for pt in range(2):
                        filler.append(
                            lambda pt=pt, xk=xk, cn=c + 1: proj_item(
                                xk, cn, pt, wk_sb, bk_sb, khT
                            )
                        )
                if pending is not None:
                    pc = pending[0]
                    for ib in range(4 * pc, 4 * pc + 4):
                        filler.append(lambda ib=ib, pc=pc: d_item(pc, ib))
                if c + 1 < CH:
                    for j, sb in enumerate(range(4 * (c + 1), 4 * (c + 1) + 4)):
                        filler.append(
                            lambda j=j, sb=sb, vts=vts: vproj_item(vts[j], sb)
                        )
                state = attn_chunk(c, filler)
                pending = (c, *state)

            # ---- epilogue: last chunk's normalize + output projection ----
            pc, pxas, prec16 = pending
            for r in range(4):
                norm_item(pc, r, pxas, prec16)
            for ib in range(4 * pc, 4 * pc + 4):
                d_item(pc, ib)

    nc.compile()
    _cache["nc"] = nc
    return nc


def _wlayout(wT):
    # [D, PL] -> SBUF layout [128, KT, PL]
    return np.ascontiguousarray(wT.reshape(KT, 128, PL).transpose(1, 0, 2)).astype(NP_DT)


def _flayout(wT):
    # [PL, D] -> SBUF layout [128, 2, D]
    return np.ascontiguousarray(wT.reshape(2, 128, D).transpose(1, 0, 2)).astype(NP_DT)


def _qk_chunks(x):
    # [S, D] -> [CH, 128 part(d%128), KT(d//128), 512(s in chunk)]
    return np.ascontiguousarray(
        x.reshape(CH, 512, KT, 128).transpose(0, 3, 2, 1)
    ).astype(NP_DT)


def _v_blocks(x):
    # [S, D] -> [SB, 128 part(d%128), KT(d//128), 128(s in block)]
    return np.ascontiguousarray(
        x.reshape(SB, 128, KT, 128).transpose(0, 3, 2, 1)
    ).astype(NP_DT)


def make_in_maps(q, k, v, mask, Wq, bq, Wk, bk, Wv, bv, Wf, bf):
    scale = 1.0 / np.sqrt(np.float32(HD))
    f32 = np.float32
    m = np.asarray(mask[0, 0])
    # lhsT for the mask preload: lhsT.T @ I == maskT_add[k, q]
    tri = np.where(m[:128, :128] == 0, f32(-1e9), f32(0.0)).astype(NP_DT)
    idn = np.eye(128, dtype=NP_DT)
    sel = np.zeros((4, 4 * HD), np.float16)
    for r in range(4):
        sel[r, HD * r : HD * (r + 1)] = 1.0
    in_maps = []
    for c in range(8):
        b, g = c // 4, c % 4
        sl = slice(g * PL, (g + 1) * PL)
        in_maps.append(
            {
                "qc": _qk_chunks(np.asarray(q[b]).astype(f32) * scale),
                "kc": _qk_chunks(np.asarray(k[b])),
                "vc": _v_blocks(np.asarray(v[b])),
                "wq": _wlayout(np.asarray(Wq)[sl, :].T),
                "wk": _wlayout(np.asarray(Wk)[sl, :].T),
                "wv": _wlayout(np.asarray(Wv)[sl, :].T),
                "wf": _flayout(np.asarray(Wf)[:, sl].T),
                "bq2": np.ascontiguousarray((np.asarray(bq)[sl] * scale).astype(f32).reshape(2, 128).T),
                "bk2": np.ascontiguousarray(np.asarray(bk)[sl].astype(f32).reshape(2, 128).T),
                "bv1": np.asarray(bv)[sl].astype(f32).reshape(1, PL),
                "bf1": (np.asarray(bf).astype(f32) / 4.0).reshape(1, D),
                "tri": tri,
                "idn": idn,
                "sel": sel,
            }
        )
    return in_maps


def _mask_is_causal(mask):
    m = np.asarray(mask[0, 0])
    return bool(np.array_equal(m != 0, np.tril(np.ones((S, S), bool))))


def _numpy_fallback(q, k, v, mask, Wq, bq, Wk, bk, Wv, bv, Wf, bf):
    out = np.empty((B, S, D), np.float32)
    m = np.asarray(mask[0, 0])
    for b in range(B):
        qh = (np.asarray(q[b]) @ np.asarray(Wq).T + bq).reshape(S, H, HD)
        kh = (np.asarray(k[b]) @ np.asarray(Wk).T + bk).reshape(S, H, HD)
        vh = (np.asarray(v[b]) @ np.asarray(Wv).T + bv).reshape(S, H, HD)
        x = np.empty((S, H, HD), np.float32)
        for hh in range(H):
            sc = qh[:, hh] @ kh[:, hh].T / np.sqrt(np.float32(HD))
            sc = np.where(m == 0, np.float32(-1e9), sc)
            sc = sc - sc.max(-1, keepdims=True)
            e = np.exp(sc)
            x[:, hh] = (e / e.sum(-1, keepdims=True)) @ vh[:, hh]
        out[b] = x.reshape(S, D) @ np.asarray(Wf).T + bf
    return out


def kernel(q, k, v, mask, Wq, bq, Wk, bk, Wv, bv, Wf, bf):
    global last_results
    if not _mask_is_causal(mask):
        return _numpy_fallback(q, k, v, mask, Wq, bq, Wk, bk, Wv, bv, Wf, bf)
    nc = build_program()
    in_maps = make_in_maps(q, k, v, mask, Wq, bq, Wk, bk, Wv, bv, Wf, bf)
    res = run_bass_kernel_spmd(nc, in_maps, core_ids=list(range(8)))
    last_results = res
    out = np.zeros((B, S, D), np.float32)
    for c in range(8):
        out[c // 4] += res.results[c]["out"].astype(np.float32)
    return out


# revision 20
# speedup vs baseline: 1.1542x; 1.0350x over previous
"""Multi-head attention (B=2, S=2048, D=1024, H=16) on 8 TRN2 NeuronCores.

Sharding: batch x head-group. Core c handles batch b=c//4 and heads
[4g, 4g+4) with g=c%4 (column-parallel QKV projections, row-parallel
output projection). Each core emits a partial [S, D] output; the host
sums the 4 partials per batch (the row-parallel all-reduce).

Chunk-major pipeline (v2): the sequence is processed in 4 chunks of 512
queries. Per chunk: project q/k for that chunk (+v for its key blocks),
run both head-pairs' scores->exp->AV, normalize, and do the output
projection — so softmax normalization and the out-projection of chunk c
hide under the attention of chunk c+1 instead of forming a serial tail.
Host re-layouts q/k/v so every chunk's input is one contiguous DMA.

Other key choices (all matmuls bf16 with f32 PSUM accum):
- Scores are computed transposed (scoresT[k, q]); the K=64 head-pair
  matmuls run concurrently in the PE array via base-partition row
  tiling. vh carries a ones column so attention row-sums fall out of
  the AV matmul.
- Causal masking of diagonal blocks is a PE preload: a [128,128]
  additive -1e9 matmul with start=True, which the score matmul then
  accumulates onto (cols beyond 128 are overwritten since their
  has_written bits stay clear) — no VectorE masking pass.
- Softmax renormalization: row sums gathered to a [4,512] tile, 1/x on
  ScalarE as exp(-ln(x)) (both fns in one ACT table set), broadcast
  across partitions by a tiny one-hot fp16 matmul, applied by VectorE.
- ~80 junk warmup matmuls at t=0 keep the PE HAM clock-gate warm while
  the first input DMAs land.
"""

import os
import numpy as np
import ml_dtypes

import concourse.bass as bass
import concourse.tile as tile
from concourse import bacc, mybir
from concourse.bass_utils import run_bass_kernel_spmd

B, S, D, H = 2, 2048, 1024, 16
HD = D // H          # 64
HL = H // 4          # 4 heads per core
PL = HL * HD         # 256 local projection dim
KT = D // 128        # 8 contraction blocks
SB = S // 128        # 16 sequence blocks of 128
CH = S // 512        # 4 sequence chunks of 512
F32 = mybir.dt.float32
F16 = mybir.dt.float16
DT = mybir.dt.bfloat16
NP_DT = ml_dtypes.bfloat16
USE_ACT_RECIP = False  # Ln+Exp thrash ACT table sets (9 loads); DVE recip hides under the skew

_cache = {}
last_results = None


def build_program():
    if "nc" in _cache:
        return _cache["nc"]
    nc = bacc.Bacc("TRN2", target_bir_lowering=False, debug=False, num_devices=8)

    # inputs, host-relaid so every DMA is contiguous with >=2KB/partition
    qc_d = nc.dram_tensor("qc", [CH, 128, KT, 512], DT, kind="ExternalInput")
    kc_d = nc.dram_tensor("kc", [CH, 128, KT, 512], DT, kind="ExternalInput")
    vc_d = nc.dram_tensor("vc", [SB, 128, KT, 128], DT, kind="ExternalInput")
    wq_d = nc.dram_tensor("wq", [128, KT, PL], DT, kind="ExternalInput")
    wk_d = nc.dram_tensor("wk", [128, KT, PL], DT, kind="ExternalInput")
    wv_d = nc.dram_tensor("wv", [128, KT, PL], DT, kind="ExternalInput")
    wf_d = nc.dram_tensor("wf", [128, 2, D], DT, kind="ExternalInput")
    bq_d = nc.dram_tensor("bq2", [128, 2], F32, kind="ExternalInput")
    bk_d = nc.dram_tensor("bk2", [128, 2], F32, kind="ExternalInput")
    bv_d = nc.dram_tensor("bv1", [1, PL], F32, kind="ExternalInput")
    bf_d = nc.dram_tensor("bf1", [1, D], F32, kind="ExternalInput")
    tri_d = nc.dram_tensor("tri", [128, 128], DT, kind="ExternalInput")
    idn_d = nc.dram_tensor("idn", [128, 128], DT, kind="ExternalInput")
    sel_d = nc.dram_tensor("sel", [4, 4 * HD], F16, kind="ExternalInput")
    out_d = nc.dram_tensor("out", [S, D], DT, kind="ExternalOutput")

    ADD = mybir.AluOpType.add
    MUL = mybir.AluOpType.mult
    EXP = mybir.ActivationFunctionType.Exp
    LN = mybir.ActivationFunctionType.Ln

    with tile.TileContext(nc) as tc:
        with (
            tc.tile_pool(name="singles", bufs=1) as singles,
            tc.tile_pool(name="qk", bufs=3) as qkp,
            tc.tile_pool(name="vin", bufs=6) as vinp,
            tc.tile_pool(name="epool", bufs=6) as epool,
            tc.tile_pool(name="apool", bufs=6) as apool,
            tc.tile_pool(name="npool", bufs=2) as npool,
            tc.tile_pool(name="opool", bufs=2) as opool,
            tc.tile_pool(name="psum", bufs=2, space="PSUM") as psum,
        ):
            wq_sb = singles.tile([128, KT, PL], DT)
            wk_sb = singles.tile([128, KT, PL], DT)
            wv_sb = singles.tile([128, KT, PL], DT)
            wf_sb = singles.tile([128, 2, D], DT)
            bq_sb = singles.tile([128, 2], F32)
            bk_sb = singles.tile([128, 2], F32)
            bv_bc = singles.tile([128, PL], F32)
            bf_bc = singles.tile([128, D], F32)
            tri_sb = singles.tile([128, 128], DT)
            idn_sb = singles.tile([128, 128], DT)
            sel_sb = singles.tile([4, 4 * HD], F16)

            qhT = singles.tile([128, 2, S], DT)   # [p within pair, pair, s]
            khT = singles.tile([128, 2, S], DT)
            vh = singles.tile([128, SB, HL, HD + 1], DT)  # [s in blk, sblk, h, hd|1]
            xn = singles.tile([128, 2, S], DT)    # normalized attn out
            nc.vector.memset(vh[:, :, :, HD : HD + 1], 1.0)

            # warmup matmuls: keep the PE HAM activity window busy while
            # the first input DMAs land, so projections start at 2.4 GHz
            wu = singles.tile([128, 128], DT)
            nc.vector.memset(wu, 0.0)
            for _ in range(80):
                wp = psum.tile([128, 128], F32, tag="C", bufs=2, name="wu")
                nc.tensor.matmul(wp, lhsT=wu, rhs=wu, start=True, stop=True)

            # weights first (small), then per-chunk inputs
            nc.sync.dma_start(wq_sb, wq_d.ap())
            nc.sync.dma_start(bq_sb, bq_d.ap())
            nc.sync.dma_start(wk_sb, wk_d.ap())
            nc.sync.dma_start(bk_sb, bk_d.ap())
            nc.sync.dma_start(wv_sb, wv_d.ap())
            nc.sync.dma_start(tri_sb, tri_d.ap())
            nc.sync.dma_start(idn_sb, idn_d.ap())
            nc.sync.dma_start(bv_bc, bv_d.ap().to_broadcast([128, PL]))

            # DMA issue is decoupled from compute: inputs for chunk c+1 are
            # DMA'd at the start of chunk c, and their projections are
            # emitted as PE "filler" between attention blocks of chunk c
            # (the PE queue is strictly in-order, so without filler it
            # would idle every block waiting on ScalarE's exp).
            def issue_qk(c):
                xq = qkp.tile([128, KT, 512], DT, tag="xin")
                nc.sync.dma_start(xq, qc_d.ap()[c])
                xk = qkp.tile([128, KT, 512], DT, tag="xin")
                nc.sync.dma_start(xk, kc_d.ap()[c])
                return xq, xk

            def issue_v(c):
                vts = []
                for sb in range(4 * c, 4 * c + 4):
                    vt = vinp.tile([128, KT, 128], DT, tag="vin", bufs=8)
                    nc.sync.dma_start(vt, vc_d.ap()[sb])
                    vts.append(vt)
                return vts

            def proj_item(xt, c, pt, w_sb, b_sb, out_sb):
                pp = psum.tile([128, 512], F32, tag="C", bufs=2, name="pp")
                for kk in range(KT):
                    nc.tensor.matmul(
                        pp,
                        lhsT=w_sb[:, kk, 128 * pt : 128 * (pt + 1)],
                        rhs=xt[:, kk, :],
                        start=(kk == 0),
                        stop=(kk == KT - 1),
                    )
                nc.vector.tensor_scalar_add(
                    out_sb[:, pt, 512 * c : 512 * (c + 1)],
                    pp,
                    b_sb[:, pt : pt + 1],
                )

            def vproj_item(vt, sb):
                pv = psum.tile([128, PL], F32, tag="C", bufs=2, name="pv")
                for kk in range(KT):
                    nc.tensor.matmul(
                        pv,
                        lhsT=vt[:, kk, :],
                        rhs=wv_sb[:, kk, :],
                        start=(kk == 0),
                        stop=(kk == KT - 1),
                    )
                nc.vector.tensor_tensor(
                    out=vh[:, sb, :, 0:HD],
                    in0=pv.rearrange("p (h e) -> p h e", h=HL),
                    in1=bv_bc.rearrange("p (h e) -> p h e", h=HL),
                    op=ADD,
                )

            def norm_item(c, r, xas, rec16, krows=4):
                # rec16 has `krows` partitions; row r % krows holds head r's
                # 1/rowsum. sel's one-hot layout makes any [krows, 64] slice
                # at column 64r a valid broadcast selector.
                hb, par = r // 2, r % 2
                rb = psum.tile([HD, 512], F32, tag="C", bufs=2, name="rb")
                nc.tensor.matmul(
                    rb,
                    lhsT=sel_sb[0:krows, HD * (r % krows) : HD * (r % krows + 1)],
                    rhs=rec16,
                    start=True,
                    stop=True,
                )
                xt_n = npool.tile([HD, 512], DT, tag="xtn", bufs=4)
                nc.vector.tensor_tensor(out=xt_n, in0=xas[r][0:HD, :], in1=rb, op=MUL)
                nc.sync.dma_start(
                    xn[64 * par : 64 * par + 64, hb, 512 * c : 512 * (c + 1)], xt_n
                )

            def d_item(c, ib):
                ob = opool.tile([128, D], DT, tag="ob")
                for oc in range(2):
                    po = psum.tile([128, 512], F32, tag="C", bufs=2, name="po")
                    for t in range(2):
                        nc.tensor.matmul(
                            po,
                            lhsT=xn[:, t, 128 * ib : 128 * (ib + 1)],
                            rhs=wf_sb[:, t, 512 * oc : 512 * (oc + 1)],
                            start=(t == 0),
                            stop=(t == 1),
                        )
                    nc.vector.tensor_tensor(
                        out=ob[:, 512 * oc : 512 * (oc + 1)],
                        in0=po,
                        in1=bf_bc[:, 512 * oc : 512 * (oc + 1)],
                        op=ADD,
                    )
                nc.sync.dma_start(out_d.ap()[128 * ib : 128 * (ib + 1), :], ob)

            def attn_chunk(c, filler, split_norm=False):
                """Attention for chunk c, interleaving `filler` PE work into
                the exp-latency gaps. AV(bj) is emitted one block late so
                scores(bj+1)+filler cover exp(bj)'s ScalarE latency.
                split_norm (last chunk): reciprocal runs per head-pair so
                hb0's normalize joins the filler during hb1's attention."""
                xas = {}
                recs = {}
                sums = npool.tile([4, 512], F32, tag="sums", bufs=3)
                nbj = 4 * c + 4
                slots = 2 * nbj

                def pop_filler(n=None):
                    # floor pacing leaves a remainder that flushes at the
                    # hb/chunk boundaries — exactly where the PE would
                    # otherwise idle on psp-slot waits and HAM rethrottles
                    nonlocal slots
                    if n is None:
                        n = len(filler) // slots if slots > 0 else 0
                    for _ in range(min(n, len(filler))):
                        filler.pop(0)()
                    slots = max(slots - 1, 0)

                for hb in range(2):
                    pxs = [
                        psum.tile([128, 512], F32, tag="B", bufs=2, name=f"px{p_}")
                        for p_ in range(2)
                    ]
                    pend_av = None
                    for bj in range(nbj):
                        band = bj >= 4 * c
                        i0 = 128 * bj if band else 512 * c
                        w = 512 * (c + 1) - i0
                        o = i0 - 512 * c
                        psp = psum.tile([128, 2, 512], F32, tag="A", bufs=2)
                        for par in range(2):
                            nc.tensor.matmul(
                                psp[:, par, 0:w],
                                lhsT=khT[
                                    64 * par : 64 * par + 64,
                                    hb,
                                    128 * bj : 128 * (bj + 1),
                                ],
                                rhs=qhT[64 * par : 64 * par + 64, hb, i0 : i0 + w],
                                start=True,
                                stop=not band,
                            )
                        if band:
                            # causal masking on the PE: accumulate a -1e9
                            # upper-triangle matmul onto the diagonal
                            # 128x128 sub-block (cols 0:128)
                            for par in range(2):
                                nc.tensor.matmul(
                                    psp[:, par, 0:128],
                                    lhsT=tri_sb,
                                    rhs=idn_sb,
                                    start=False,
                                    stop=True,
                                )
                        et = epool.tile([128, 2, 512], DT, tag="et")
                        nc.scalar.activation(et[:, :, 0:w], psp[:, :, 0:w], EXP)
                        if pend_av is not None:
                            pop_filler()
                            pend_av()
                        pend_av = (
                            lambda et=et, w=w, o=o, bj=bj: [
                                nc.tensor.matmul(
                                    pxs[par][0 : HD + 1, o : o + w],
                                    lhsT=vh[:, bj, 2 * hb + par, :],
                                    rhs=et[:, par, 0:w],
                                    start=(bj == 0),
                                    stop=(bj == nbj - 1),
                                )
                                for par in range(2)
                            ]
                        )
                    pop_filler()
                    pend_av()
                    pop_filler(2)  # boundary flush: cover the psp-slot wait
                    if split_norm:
                        sums_h = npool.tile([2, 512], F32, tag="sums2", name="sums_h")
                    else:
                        sums_h = sums
                    for par in range(2):
                        r = 2 * hb + par
                        xa = apool.tile([HD + 1, 512], F32, tag="xa", bufs=10)
                        nc.vector.tensor_copy(out=xa, in_=pxs[par][0 : HD + 1, :])
                        row = par if split_norm else r
                        nc.sync.dma_start(sums_h[row : row + 1, :], xa[HD : HD + 1, :])
                        xas[r] = xa
                    if split_norm:
                        rec = npool.tile([2, 512], F32, tag="rec2")
                        nc.vector.reciprocal(rec, sums_h)
                        r16 = npool.tile([2, 512], F16, tag="rec16b")
                        nc.vector.tensor_copy(r16, rec)
                        recs[hb] = r16
                        if hb == 0:
                            for r in range(2):
                                filler.append(
                                    lambda r=r, r16=r16: norm_item(
                                        c, r, xas, r16, krows=2
                                    )
                                )
                if split_norm:
                    # drain leftover filler (hb1's rec chain resolves under it)
                    for f in filler:
                        f()
                    return xas, recs[1]
                rec16 = npool.tile([4, 512], F16, tag="rec16")
                rec = npool.tile([4, 512], F32, tag="rec")
                nc.vector.reciprocal(rec, sums)
                nc.vector.tensor_copy(rec16, rec)
                # drain any leftover filler
                for f in filler:
                    f()
                return xas, rec16

            # ---- prologue: chunk 0 inputs + projections, serial ----
            xq0, xk0 = issue_qk(0)
            vts0 = issue_v(0)
            for pt in range(2):
                proj_item(xq0, 0, pt, wq_sb, bq_sb, qhT)
            for pt in range(2):
                proj_item(xk0, 0, pt, wk_sb, bk_sb, khT)
            for j, sb in enumerate(range(0, 4)):
                vproj_item(vts0[j], sb)

            pending = None
            for c in range(CH):
                filler = []
                if c + 1 < CH:
                    xq, xk = issue_qk(c + 1)
                    vts = issue_v(c + 1)
                    if c == 0:
                        nc.sync.dma_start(wf_sb, wf_d.ap())
                        nc.sync.dma_start(bf_bc, bf_d.ap().to_broadcast([128, D]))
                        nc.sync.dma_start(sel_sb, sel_d.ap())
                if pending is not None:
                    pc, pxas, prec16 = pending
                    for r in range(4):
                        filler.append(
                            lambda r=r: norm_item(pc, r, pxas, prec16)
                        )
                if c + 1 < CH:
                    for pt in range(2):
                        filler.append(
                            lambda pt=pt, xq=xq, cn=c + 1: proj_item(
                                xq, cn, pt, wq_sb, bq_sb, qhT
                            )
                        )
                    for pt in range(2):
                        filler.append(
                            lambda pt=pt, xk=xk, cn=c + 1: proj_item(
                                xk, cn, pt, wk_sb, bk_sb, khT
                            )
                        )
                if pending is not None:
                    pc = pending[0]
                    for ib in range(4 * pc, 4 * pc + 4):
                        filler.append(lambda ib=ib, pc=pc: d_item(pc, ib))
                if c + 1 < CH:
                    for j, sb in enumerate(range(4 * (c + 1), 4 * (c + 1) + 4)):
                        filler.append(
                            lambda j=j, sb=sb, vts=vts: vproj_item(vts[j], sb)
                        )
                state = attn_chunk(c, filler, split_norm=(c == CH - 1))
                pending = (c, *state)

            # ---- epilogue: last chunk's normalize + output projection ----
            # junk matmuls keep the PE HAM-warm while the reciprocal chain
            # (xa copy -> sums DMA -> DVE recip -> cast) resolves; they have
            # no deps so they fill the otherwise-idle window
            for _ in range(48):
                wp = psum.tile([128, 128], F32, tag="C", bufs=2, name="wu")
                nc.tensor.matmul(wp, lhsT=wu, rhs=wu, start=True, stop=True)
            pc, pxas, prec16 = pending
            for r in (2, 3):  # r=0,1 were filler during the last chunk's hb1
                norm_item(pc, r, pxas, prec16, krows=2)
            for ib in range(4 * pc, 4 * pc + 4):
                d_item(pc, ib)

    nc.compile()
    _cache["nc"] = nc
    return nc


def _wlayout(wT):
    # [D, PL] -> SBUF layout [128, KT, PL]
    return np.ascontiguousarray(wT.reshape(KT, 128, PL).transpose(1, 0, 2)).astype(NP_DT)


def _flayout(wT):
    # [PL, D] -> SBUF layout [128, 2, D]
    return np.ascontiguousarray(wT.reshape(2, 128, D).transpose(1, 0, 2)).astype(NP_DT)


def _qk_chunks(x):
    # [S, D] -> [CH, 128 part(d%128), KT(d//128), 512(s in chunk)]
    return np.ascontiguousarray(
        x.reshape(CH, 512, KT, 128).transpose(0, 3, 2, 1)
    ).astype(NP_DT)


def _v_blocks(x):
    # [S, D] -> [SB, 128 part(d%128), KT(d//128), 128(s in block)]
    return np.ascontiguousarray(
        x.reshape(SB, 128, KT, 128).transpose(0, 3, 2, 1)
    ).astype(NP_DT)


def make_in_maps(q, k, v, mask, Wq, bq, Wk, bk, Wv, bv, Wf, bf):
    scale = 1.0 / np.sqrt(np.float32(HD))
    f32 = np.float32
    m = np.asarray(mask[0, 0])
    # lhsT for the mask preload: lhsT.T @ I == maskT_add[k, q]
    tri = np.where(m[:128, :128] == 0, f32(-1e9), f32(0.0)).astype(NP_DT)
    idn = np.eye(128, dtype=NP_DT)
    sel = np.zeros((4, 4 * HD), np.float16)
    for r in range(4):
        sel[r, HD * r : HD * (r + 1)] = 1.0
    in_maps = []
    for c in range(8):
        b, g = c // 4, c % 4
        sl = slice(g * PL, (g + 1) * PL)
        in_maps.append(
            {
                "qc": _qk_chunks(np.asarray(q[b]).astype(f32) * scale),
                "kc": _qk_chunks(np.asarray(k[b])),
                "vc": _v_blocks(np.asarray(v[b])),
                "wq": _wlayout(np.asarray(Wq)[sl, :].T),
                "wk": _wlayout(np.asarray(Wk)[sl, :].T),
                "wv": _wlayout(np.asarray(Wv)[sl, :].T),
                "wf": _flayout(np.asarray(Wf)[:, sl].T),
                "bq2": np.ascontiguousarray((np.asarray(bq)[sl] * scale).astype(f32).reshape(2, 128).T),
                "bk2": np.ascontiguousarray(np.asarray(bk)[sl].astype(f32).reshape(2, 128).T),
                "bv1": np.asarray(bv)[sl].astype(f32).reshape(1, PL),
                "bf1": (np.asarray(bf).astype(f32) / 4.0).reshape(1, D),
                "tri": tri,
                "idn": idn,
                "sel": sel,
            }
        )
    return in_maps


def _mask_is_causal(mask):
    m = np.asarray(mask[0, 0])
    return bool(np.array_equal(m != 0, np.tril(np.ones((S, S), bool))))


def _numpy_fallback(q, k, v, mask, Wq, bq, Wk, bk, Wv, bv, Wf, bf):
    out = np.empty((B, S, D), np.float32)
    m = np.asarray(mask[0, 0])
    for b in range(B):
        qh = (np.asarray(q[b]) @ np.asarray(Wq).T + bq).reshape(S, H, HD)
        kh = (np.asarray(k[b]) @ np.asarray(Wk).T + bk).reshape(S, H, HD)
        vh = (np.asarray(v[b]) @ np.asarray(Wv).T + bv).reshape(S, H, HD)
        x = np.empty((S, H, HD), np.float32)
        for hh in range(H):
            sc = qh[:, hh] @ kh[:, hh].T / np.sqrt(np.float32(HD))
            sc = np.where(m == 0, np.float32(-1e9), sc)
            sc = sc - sc.max(-1, keepdims=True)
            e = np.exp(sc)
            x[:, hh] = (e / e.sum(-1, keepdims=True)) @ vh[:, hh]
        out[b] = x.reshape(S, D) @ np.asarray(Wf).T + bf
    return out


def kernel(q, k, v, mask, Wq, bq, Wk, bk, Wv, bv, Wf, bf):
    global last_results
    if not _mask_is_causal(mask):
        return _numpy_fallback(q, k, v, mask, Wq, bq, Wk, bk, Wv, bv, Wf, bf)
    nc = build_program()
    in_maps = make_in_maps(q, k, v, mask, Wq, bq, Wk, bk, Wv, bv, Wf, bf)
    res = run_bass_kernel_spmd(nc, in_maps, core_ids=list(range(8)))
    last_results = res
    out = np.zeros((B, S, D), np.float32)
    for c in range(8):
        out[c // 4] += res.results[c]["out"].astype(np.float32)
    return out
